# revision 6
# baseline (speedup 1.0000x reference)
"""Masked-loss kernel for nn_MLoss_9715216024200 on 8 Trainium2 NeuronCores.

loss = sum(where(y[...,0]>0.5, (y-x)^2 - a*x^2, 0)) + a*sum(x[...,0]^2)
with x,y f32 (256, 10647, 5); output is a f32 scalar.

Sharding: flatten both tensors to cells (5 contiguous f32 each), pad with
256 zero-cells (mathematically neutral: y0=0 -> mask 0, x=0 -> no bg term),
reshape to (8 cores, 128 partitions, 2662 cells).  Each core streams its
13 MiB at the ~360 GB/s HBM roofline; the three compute engines split the
per-tile work so every engine stays well under the per-tile DMA time:

  DVE : m  = bf16(y0 > 0.5)                    [cells]
        d  = y - x (bf16 out)                  [5*cells]
        dm = d * m (stride-0 broadcast of m)   [5*cells] -> head of dmx
  Pool: xm  = x * m (broadcast)                [5*cells]
        xs0 = bf16(sqrt(a)*x0)                 [cells]   -> tail of dmx
  ACT : acc1[t] = sum(dmx^2) = sum((m*(y-x))^2) + a*sum(x0^2)
        acc2[t] = sum(xm^2)                    (no scale: host applies a)

m*v^2 == (m*v)^2 because m is 0/1.  The mask is read through a stride-0
broadcast AP and never materialized at feature width.

The tile sizes telescope down at the end (geometric, ratio ~1.7) so that
each tile's accumulate lands before the remaining DMA stream finishes;
the last TAIL_DVE tiles' reductions run as DVE scalar_tensor_tensor
accumulates (no ACT accumulator-read latency) so the final chain after
the last byte is just d -> dm -> stt on an otherwise idle DVE.

Host combines: total = sum(acc1) - ALPHA*sum(acc2), in f64 over
8 cores x 128 partitions x N_TILES tiles.
"""
import sys

for _p in ('/opt/trn_rl_repo',):
    if _p in sys.path:
        sys.path.remove(_p)
    sys.path.insert(0, _p)

import numpy as np

B, C, F = 256, 10647, 5
THRESH = 0.5
ALPHA = 0.1
N_CORES = 8
P = 128
CELLS = B * C                      # 2,725,632
CELLS_PER_PART = 2662              # ceil to 8*128*2662 = 2,725,888
PAD_CELLS = N_CORES * P * CELLS_PER_PART - CELLS   # 256
FD = CELLS_PER_PART * F            # 13310 elems per partition per core

import os as _os
_ts = _os.environ.get('TILE_SIZES', '')
TILE_SIZES = ([int(v) for v in _ts.split(',')] if _ts
              else [378] * 6 + [190, 110, 64, 30])
assert sum(TILE_SIZES) == CELLS_PER_PART
N_TILES = len(TILE_SIZES)
# last k tiles: both reductions on DVE stt
TAIL_DVE = int(_os.environ.get('TAIL_DVE', '1'))
BUFS = [int(v) for v in _os.environ.get('BUFS', '6,6,6,2').split(',')]

_compiled = None


def _build():
    from contextlib import ExitStack
    import concourse.tile as tile
    from concourse import bacc, mybir

    sqa = float(np.sqrt(ALPHA))

    nc = bacc.Bacc("TRN2", target_bir_lowering=False, debug=False,
                   enable_asserts=True, num_devices=N_CORES)
    x_d = nc.dram_tensor("x", [P, FD], mybir.dt.float32, kind="ExternalInput").ap()
    y_d = nc.dram_tensor("y", [P, FD], mybir.dt.float32, kind="ExternalInput").ap()
    o_d = nc.dram_tensor("o", [P, 2 * N_TILES], mybir.dt.float32,
                         kind="ExternalOutput").ap()

    f32 = mybir.dt.float32
    bf16 = mybir.dt.bfloat16
    Sq = mybir.ActivationFunctionType.Square
    Alu = mybir.AluOpType

    with tile.TileContext(nc) as tc, ExitStack() as ctx:
        xp = ctx.enter_context(tc.tile_pool(name="x", bufs=BUFS[0]))
        yp = ctx.enter_context(tc.tile_pool(name="y", bufs=BUFS[1]))
        wp = ctx.enter_context(tc.tile_pool(name="work", bufs=BUFS[2]))
        sp = ctx.enter_context(tc.tile_pool(name="scratch", bufs=BUFS[3]))
        ap_ = ctx.enter_context(tc.tile_pool(name="acc", bufs=1))

        # interleaved acc layout: columns [2t, 2t+1] = (dm-side, xm-side)
        acc = ap_.tile([P, 2 * N_TILES], f32)

        tail = []
        off = 0
        for t, cells in enumerate(TILE_SIZES):
            fd = cells * F
            is_tail = t >= N_TILES - TAIL_DVE
            yt = yp.tile([P, fd], f32, tag="yt")
            xt = xp.tile([P, fd], f32, tag="xt")
            sl = slice(off, off + fd)
            off += fd
            nc.sync.dma_start(yt[:], y_d[:, sl])
            nc.sync.dma_start(xt[:], x_d[:, sl])

            # DVE: per-cell mask (bf16 0/1), read by ym/xm as broadcast view
            m = wp.tile([P, cells], bf16, tag="m")
            nc.vector.tensor_scalar(m[:], yt[:, 0::F], THRESH, None,
                                    op0=Alu.is_gt)
            mv = m[:].unsqueeze(2).broadcast_to((P, cells, F))

            # dmx = [dm (fd) | xs0 (cells)]: one fused Square+accum covers
            # sum((m*(y-x))^2) + a*sum(x0^2)
            dmx = wp.tile([P, fd + cells], bf16, tag="dmx")

            dt_ = wp.tile([P, fd], bf16, tag="d")
            nc.vector.tensor_tensor(dt_[:], yt[:], xt[:], op=Alu.subtract)
            nc.vector.tensor_tensor(
                dmx[:, 0:fd].rearrange("p (k f) -> p k f", f=F),
                dt_[:].rearrange("p (k f) -> p k f", f=F), mv, op=Alu.mult)

            # Pool: xs0 into the dmx tail (gates the dmx reduction), then xm
            nc.gpsimd.tensor_scalar(dmx[:, fd:fd + cells], xt[:, 0::F],
                                    sqa, None, op0=Alu.mult)
            xmt = wp.tile([P, fd], bf16, tag="xm")
            (nc.vector if is_tail else nc.gpsimd).tensor_tensor(
                xmt[:].rearrange("p (k f) -> p k f", f=F),
                xt[:].rearrange("p (k f) -> p k f", f=F), mv, op=Alu.mult)

            if is_tail:
                tail.append((t, dmx, xmt, cells))
            else:
                sq = sp.tile([P, fd + cells], bf16, tag="sq")
                nc.scalar.activation(sq[:], dmx[:], Sq,
                                     accum_out=acc[:, 2 * t:2 * t + 1])
                sq2 = sp.tile([P, fd], bf16, tag="sq2")
                nc.scalar.activation(sq2[:], xmt[:], Sq,
                                     accum_out=acc[:, 2 * t + 1:2 * t + 2])

        for (t, dmx, xmt, cells) in tail:
            # deferred past the loop so the last tiles' ym/xm/dm (which gate
            # these) run first on DVE
            fd = cells * F
            s1 = sp.tile([P, fd + cells], bf16, tag="sq")
            nc.vector.scalar_tensor_tensor(
                s1[:], dmx[:], 1.0, dmx[:],
                op0=Alu.mult, op1=Alu.mult, accum_out=acc[:, 2 * t:2 * t + 1])
            s2 = sp.tile([P, fd], bf16, tag="sq2")
            nc.vector.scalar_tensor_tensor(
                s2[:], xmt[:], 1.0, xmt[:],
                op0=Alu.mult, op1=Alu.mult,
                accum_out=acc[:, 2 * t + 1:2 * t + 2])

        nc.sync.dma_start(o_d[:], acc[:])

    nc.compile()
    return nc


def _shard(a: np.ndarray) -> list[np.ndarray]:
    flat = a.reshape(-1)
    pad = np.zeros(PAD_CELLS * F, dtype=a.dtype)
    flat = np.concatenate([flat, pad])
    per_core = flat.reshape(N_CORES, P, FD)
    return [np.ascontiguousarray(per_core[i]) for i in range(N_CORES)]


def kernel(x: np.ndarray, y: np.ndarray) -> np.ndarray:
    global _compiled
    if _compiled is None:
        _compiled = _build()
    nc = _compiled

    from concourse.bass_utils import run_bass_kernel_spmd

    xs = _shard(np.asarray(x, dtype=np.float32))
    ys = _shard(np.asarray(y, dtype=np.float32))
    in_maps = [{"x": xs[i], "y": ys[i]} for i in range(N_CORES)]
    res = run_bass_kernel_spmd(nc, in_maps, core_ids=list(range(N_CORES)))

    total = np.float64(0.0)
    for r in res.results:
        o = r["o"].astype(np.float64)
        total += o[:, 0::2].sum()
        total -= ALPHA * o[:, 1::2].sum()
    return np.float32(total)


# revision 7
# speedup vs baseline: 1.0465x; 1.0465x over previous
"""Masked-loss kernel for nn_MLoss_9715216024200 on 8 Trainium2 NeuronCores.

loss = sum(where(y[...,0]>0.5, (y-x)^2 - a*x^2, 0)) + a*sum(x[...,0]^2)
with x,y f32 (256, 10647, 5); output is a f32 scalar.

Sharding: flatten both tensors to cells (5 contiguous f32 each), pad with
256 zero-cells (mathematically neutral: y0=0 -> mask 0, x=0 -> no bg term),
reshape to (8 cores, 128 partitions, 2662 cells).  Each core streams its
13 MiB at the ~360 GB/s HBM roofline; the three compute engines split the
per-tile work so every engine stays under the per-tile DMA time:

  DVE : m  = bf16(y0 > 0.5)                    [cells]
        d  = y - x (bf16 out)                  [5*cells]
        dm = d * m (stride-0 broadcast of m)   [5*cells] -> into group dmx
  Pool: xs0 = bf16(sqrt(a)*x0)                 [cells]   -> into group dmx
        xm  = x * m (broadcast)                [5*cells] -> into group xmg
  ACT : acc1[g] = sum(dmx^2) = sum((m*(y-x))^2) + a*sum(x0^2)
        acc2[g] = sum(xmg^2)                   (no scale: host applies a)

m*v^2 == (m*v)^2 because m is 0/1.  The mask is read through a stride-0
broadcast AP and never materialized at feature width.

DVE/Pool work per DMA tile, but ACT reduces GROUPS of consecutive tiles
(their dm/xs0/xm land in one contiguous group buffer): the ~740ns fixed
cost per accumulate (187ns accumulator read + SBUF-access init) is paid
per group, not per tile.  The tile sizes telescope at the end so only a
tiny chain remains after the last DMA; the last tiles' reductions run as
DVE scalar_tensor_tensor accumulates on an otherwise-idle DVE.

Host combines: total = sum(acc1) - ALPHA*sum(acc2), in f64.
"""
import sys

for _p in ('/opt/trn_rl_repo',):
    if _p in sys.path:
        sys.path.remove(_p)
    sys.path.insert(0, _p)

import numpy as np

B, C, F = 256, 10647, 5
THRESH = 0.5
ALPHA = 0.1
N_CORES = 8
P = 128
CELLS = B * C                      # 2,725,632
CELLS_PER_PART = 2662              # ceil to 8*128*2662 = 2,725,888
PAD_CELLS = N_CORES * P * CELLS_PER_PART - CELLS   # 256
FD = CELLS_PER_PART * F            # 13310 elems per partition per core

import os as _os
# groups of DMA-tile sizes; each group gets one pair of accumulates
_gs = _os.environ.get('GROUPS', '')
GROUPS = ([[int(v) for v in g.split(',')] for g in _gs.split('/')] if _gs
          else [[222, 222]] * 5 + [[160], [120], [82], [50], [30]])
TILE_SIZES = [c for g in GROUPS for c in g]
assert sum(TILE_SIZES) == CELLS_PER_PART, sum(TILE_SIZES)
N_GROUPS = len(GROUPS)
# last k groups: reductions on DVE via scalar_tensor_tensor
TAIL_DVE = int(_os.environ.get('TAIL_DVE', '2'))
BUFS = [int(v) for v in _os.environ.get('BUFS', '6,6,4,4').split(',')]

_compiled = None


def _build():
    from contextlib import ExitStack
    import concourse.tile as tile
    from concourse import bacc, mybir

    sqa = float(np.sqrt(ALPHA))

    nc = bacc.Bacc("TRN2", target_bir_lowering=False, debug=False,
                   enable_asserts=True, num_devices=N_CORES)
    x_d = nc.dram_tensor("x", [P, FD], mybir.dt.float32, kind="ExternalInput").ap()
    y_d = nc.dram_tensor("y", [P, FD], mybir.dt.float32, kind="ExternalInput").ap()
    o_d = nc.dram_tensor("o", [P, 2 * N_GROUPS], mybir.dt.float32,
                         kind="ExternalOutput").ap()

    f32 = mybir.dt.float32
    bf16 = mybir.dt.bfloat16
    Sq = mybir.ActivationFunctionType.Square
    Alu = mybir.AluOpType

    with tile.TileContext(nc) as tc, ExitStack() as ctx:
        xp = ctx.enter_context(tc.tile_pool(name="x", bufs=BUFS[0]))
        yp = ctx.enter_context(tc.tile_pool(name="y", bufs=BUFS[1]))
        wp = ctx.enter_context(tc.tile_pool(name="work", bufs=BUFS[2]))
        sp = ctx.enter_context(tc.tile_pool(name="scratch", bufs=BUFS[3]))
        ap_ = ctx.enter_context(tc.tile_pool(name="acc", bufs=1))

        # acc columns [2g, 2g+1] = (dm-side, xm-side) of group g
        acc = ap_.tile([P, 2 * N_GROUPS], f32)

        tail = []
        off = 0
        for g, sizes in enumerate(GROUPS):
            gcells = sum(sizes)
            gfd = gcells * F
            # group buffers: dmx = [dm(t0)|xs0(t0)|dm(t1)|xs0(t1)|...],
            # xmg = [xm(t0)|xm(t1)|...]
            dmx = wp.tile([P, gfd + gcells], bf16, tag="dmx")
            xmg = wp.tile([P, gfd], bf16, tag="xm")
            doff = 0
            xoff = 0
            for cells in sizes:
                fd = cells * F
                yt = yp.tile([P, fd], f32, tag="yt")
                xt = xp.tile([P, fd], f32, tag="xt")
                sl = slice(off, off + fd)
                off += fd
                nc.sync.dma_start(yt[:], y_d[:, sl])
                nc.sync.dma_start(xt[:], x_d[:, sl])

                # DVE: per-cell mask, read by dm/xm as a broadcast view
                m = wp.tile([P, cells], bf16, tag="m")
                nc.vector.tensor_scalar(m[:], yt[:, 0::F], THRESH, None,
                                        op0=Alu.is_gt)
                mv = m[:].unsqueeze(2).broadcast_to((P, cells, F))

                dt_ = wp.tile([P, fd], bf16, tag="d")
                nc.vector.tensor_tensor(dt_[:], yt[:], xt[:], op=Alu.subtract)
                nc.vector.tensor_tensor(
                    dmx[:, doff:doff + fd].rearrange("p (k f) -> p k f", f=F),
                    dt_[:].rearrange("p (k f) -> p k f", f=F), mv, op=Alu.mult)

                # Pool: xs0 (gates the dmx reduction), then xm
                nc.gpsimd.tensor_scalar(dmx[:, doff + fd:doff + fd + cells],
                                        xt[:, 0::F], sqa, None, op0=Alu.mult)
                nc.gpsimd.tensor_tensor(
                    xmg[:, xoff:xoff + fd].rearrange("p (k f) -> p k f", f=F),
                    xt[:].rearrange("p (k f) -> p k f", f=F), mv, op=Alu.mult)
                doff += fd + cells
                xoff += fd

            if g >= N_GROUPS - TAIL_DVE:
                tail.append((g, dmx, xmg, gcells))
            else:
                sq = sp.tile([P, gfd + gcells], bf16, tag="sq")
                nc.scalar.activation(sq[:], dmx[:], Sq,
                                     accum_out=acc[:, 2 * g:2 * g + 1])
                sq2 = sp.tile([P, gfd], bf16, tag="sq2")
                nc.scalar.activation(sq2[:], xmg[:], Sq,
                                     accum_out=acc[:, 2 * g + 1:2 * g + 2])

        for (g, dmx, xmg, gcells) in tail:
            # deferred past the loop so the last tiles' d/dm (which gate
            # these) run first on DVE
            gfd = gcells * F
            s1 = sp.tile([P, gfd + gcells], bf16, tag="sq")
            nc.vector.scalar_tensor_tensor(
                s1[:], dmx[:], 1.0, dmx[:],
                op0=Alu.mult, op1=Alu.mult, accum_out=acc[:, 2 * g:2 * g + 1])
            s2 = sp.tile([P, gfd], bf16, tag="sq2")
            nc.vector.scalar_tensor_tensor(
                s2[:], xmg[:], 1.0, xmg[:],
                op0=Alu.mult, op1=Alu.mult,
                accum_out=acc[:, 2 * g + 1:2 * g + 2])

        nc.sync.dma_start(o_d[:], acc[:])

    nc.compile()
    return nc


def _shard(a: np.ndarray) -> list[np.ndarray]:
    flat = a.reshape(-1)
    pad = np.zeros(PAD_CELLS * F, dtype=a.dtype)
    flat = np.concatenate([flat, pad])
    per_core = flat.reshape(N_CORES, P, FD)
    return [np.ascontiguousarray(per_core[i]) for i in range(N_CORES)]


def kernel(x: np.ndarray, y: np.ndarray) -> np.ndarray:
    global _compiled
    if _compiled is None:
        _compiled = _build()
    nc = _compiled

    from concourse.bass_utils import run_bass_kernel_spmd

    xs = _shard(np.asarray(x, dtype=np.float32))
    ys = _shard(np.asarray(y, dtype=np.float32))
    in_maps = [{"x": xs[i], "y": ys[i]} for i in range(N_CORES)]
    res = run_bass_kernel_spmd(nc, in_maps, core_ids=list(range(N_CORES)))

    total = np.float64(0.0)
    for r in res.results:
        o = r["o"].astype(np.float64)
        total += o[:, 0::2].sum()
        total -= ALPHA * o[:, 1::2].sum()
    return np.float32(total)


# revision 8
# speedup vs baseline: 1.1023x; 1.0533x over previous
"""Masked-loss kernel for nn_MLoss_9715216024200 on 8 Trainium2 NeuronCores.

loss = sum(where(y[...,0]>0.5, (y-x)^2 - a*x^2, 0)) + a*sum(x[...,0]^2)
with x,y f32 (256, 10647, 5); output is a f32 scalar.

Sharding: flatten both tensors to cells (5 contiguous f32 each), pad with
256 zero-cells (mathematically neutral: y0=0 -> mask 0, x=0 -> no bg term),
reshape to (8 cores, 128 partitions, 2662 cells).  Each core streams its
13 MiB at the ~358 GB/s HBM roofline while three compute engines split the
elementwise work (each under the ~38 us DMA time):

  per tile:
    GpSimd: m5  = bf16(y0 > 0.5) replicated to all 5 features (contiguous)
            xs0 = bf16(sqrt(a)*x0)  -> tail slice of the dmx tile
    DVE:    d   = y - x   (f32 1x, bf16 out)
            dm  = d * m5  (bf16 2x) -> head slice of dmx
            xm  = x * m5  (mixed 1x, bf16 out)
    ScalarE (Square + accum_out, fp32 accumulate):
            acc1[t] = sum(dmx^2) = sum((m*d)^2) + a*sum(x0^2)
            acc2[t] = sum(xm^2)  (unscaled; host applies a)

m*v^2 == (m*v)^2 because m is 0/1, which is what lets ScalarE's fused
Square-accumulate do all reductions.  bf16 intermediates cost ~1e-6
relative error on the final sum.  Host combines:
total = sum(acc1) - a*sum(acc2), in f64 over 8 cores x 128 partitions.
"""
import sys

for _p in ('/opt/trn_rl_repo',):
    if _p in sys.path:
        sys.path.remove(_p)
    sys.path.insert(0, _p)

import numpy as np

B, C, F = 256, 10647, 5
THRESH = 0.5
ALPHA = 0.1
N_CORES = 8
P = 128
CELLS = B * C                      # 2,725,632
CELLS_PER_PART = 2662              # ceil to 8*128*2662 = 2,725,888
PAD_CELLS = N_CORES * P * CELLS_PER_PART - CELLS   # 256
FD = CELLS_PER_PART * F            # 13310 elems per partition per core
# 253-cell tiles amortize DMA and instruction overhead; the tail tiles
# shrink so the post-last-DMA dependency chain is short.
import os as _os
_ts = _os.environ.get('TILE_SIZES', '')
TILE_SIZES = ([int(v) for v in _ts.split(',')] if _ts
              else [253] * 8 + [218, 178, 121, 121])  # sums to CELLS_PER_PART
assert sum(TILE_SIZES) == CELLS_PER_PART
N_TILES = len(TILE_SIZES)
_xmp = _os.environ.get('XM_ON_POOL', '3,6,9')
XM_ON_POOL = set(int(v) for v in _xmp.split(',') if v)
TTR_TAIL = int(_os.environ.get('TTR_TAIL', '2'))  # last k tiles: sq2 on DVE
M5_ON_DVE = set(int(v) for v in _os.environ.get('M5_ON_DVE', '0').split(',') if v != '')
X0_ON_ACT = _os.environ.get('X0_ON_ACT', '0') == '1'
STORE_ON_ACT = _os.environ.get('STORE_ON_ACT', '0') == '1'
BUFS = [int(v) for v in _os.environ.get('BUFS', '8,8,8,4').split(',')]

_compiled = None


def _build():
    from contextlib import ExitStack
    import concourse.tile as tile
    from concourse import bacc, mybir

    sqa = float(np.sqrt(ALPHA))

    nc = bacc.Bacc("TRN2", target_bir_lowering=False, debug=False,
                   enable_asserts=True, num_devices=N_CORES)
    x_d = nc.dram_tensor("x", [P, FD], mybir.dt.float32, kind="ExternalInput").ap()
    y_d = nc.dram_tensor("y", [P, FD], mybir.dt.float32, kind="ExternalInput").ap()
    o_d = nc.dram_tensor("o", [P, 2 * N_TILES], mybir.dt.float32,
                         kind="ExternalOutput").ap()

    f32 = mybir.dt.float32
    bf16 = mybir.dt.bfloat16
    Sq = mybir.ActivationFunctionType.Square
    Alu = mybir.AluOpType

    with tile.TileContext(nc) as tc, ExitStack() as ctx:
        xp = ctx.enter_context(tc.tile_pool(name="x", bufs=BUFS[0]))
        yp = ctx.enter_context(tc.tile_pool(name="y", bufs=BUFS[1]))
        wp = ctx.enter_context(tc.tile_pool(name="work", bufs=BUFS[2]))
        sp = ctx.enter_context(tc.tile_pool(name="scratch", bufs=BUFS[3]))
        ap_ = ctx.enter_context(tc.tile_pool(name="acc", bufs=1))

        # interleaved acc layout: columns [2t, 2t+1] = (dm-side, xm-side) of
        # tile t, so each tile's pair can be stored as soon as it's ready
        acc = ap_.tile([P, 2 * N_TILES], f32)

        tail_ttr = []
        off = 0
        for t, cells in enumerate(TILE_SIZES):
            fd = cells * F
            xt = xp.tile([P, fd], f32, tag="xt")
            yt = yp.tile([P, fd], f32, tag="yt")
            sl = slice(off, off + fd)
            off += fd
            nc.sync.dma_start(yt[:], y_d[:, sl])
            # tile 0's x descgen on ACT's HWDGE port, parallel with y0's on SP
            (nc.scalar if t == 0 and X0_ON_ACT else nc.sync).dma_start(
                xt[:], x_d[:, sl])

            dmx = wp.tile([P, fd + cells], bf16, tag="dmx")

            # bf16 mask replicated to all 5 features (contiguous); emitted
            # before xs0 because dm (critical path) waits on it.  Tile 0's
            # mask runs on DVE: at the pipeline head Pool's slow broadcast
            # would gate the first dm (and ACT's start)
            m5 = wp.tile([P, fd], bf16, tag="m5")
            y0b = yt[:, 0::F].unsqueeze(2).broadcast_to((P, cells, F))
            m5_eng = nc.vector if t in M5_ON_DVE else nc.gpsimd
            m5_eng.tensor_scalar(
                m5[:].rearrange("p (k f) -> p k f", f=F), y0b,
                THRESH, None, op0=Alu.is_gt)

            # GpSimd: xs0 = sqrt(a)*x0 into the tail slice of dmx
            nc.gpsimd.tensor_scalar(dmx[:, fd:fd + cells], xt[:, 0::F],
                                    sqa, None, op0=Alu.mult)

            # DVE: d = y - x (bf16 out), dm = d*m5 (bf16 2x), xm = x*m5
            dt_ = wp.tile([P, fd], bf16, tag="d")
            nc.vector.tensor_tensor(dt_[:], yt[:], xt[:], op=Alu.subtract)
            nc.vector.tensor_tensor(dmx[:, 0:fd], dt_[:], m5[:], op=Alu.mult)
            xmt = wp.tile([P, fd], bf16, tag="xm")
            xm_eng = nc.gpsimd if t in XM_ON_POOL else nc.vector
            xm_eng.tensor_tensor(xmt[:], xt[:], m5[:], op=Alu.mult)

            # ScalarE: fused square + row-sum into per-tile accumulators
            sq = sp.tile([P, fd + cells], bf16, tag="sq")
            nc.scalar.activation(sq[:], dmx[:], Sq, accum_out=acc[:, 2 * t:2 * t + 1])
            if t >= N_TILES - TTR_TAIL:
                # tail: fused square+row-sum on DVE, in parallel with ACT;
                # deferred past the loop so the last tiles' dm (which gates
                # ACT) runs first on DVE
                tail_ttr.append((t, xmt, cells))
            else:
                sq2 = sp.tile([P, fd], bf16, tag="sq2")
                nc.scalar.activation(sq2[:], xmt[:], Sq,
                                     accum_out=acc[:, 2 * t + 1:2 * t + 2])

        for (t, xmt, cells) in tail_ttr:
            # xm * 1 * xm summed per row == sum(xm^2); runs on DVE
            # (scalar_tensor_tensor is Pool-invalid but DVE-valid on HW)
            sq2 = sp.tile([P, cells * F], bf16, tag="sq2")
            nc.vector.scalar_tensor_tensor(
                sq2[:], xmt[:], 1.0, xmt[:],
                op0=Alu.mult, op1=Alu.mult, accum_out=acc[:, 2 * t + 1:2 * t + 2])

        (nc.scalar if STORE_ON_ACT else nc.sync).dma_start(o_d[:], acc[:])

    nc.compile()
    return nc


def _shard(a: np.ndarray) -> list[np.ndarray]:
    flat = a.reshape(-1)
    pad = np.zeros(PAD_CELLS * F, dtype=a.dtype)
    flat = np.concatenate([flat, pad])
    per_core = flat.reshape(N_CORES, P, FD)
    return [np.ascontiguousarray(per_core[i]) for i in range(N_CORES)]


def kernel(x: np.ndarray, y: np.ndarray) -> np.ndarray:
    global _compiled
    if _compiled is None:
        _compiled = _build()
    nc = _compiled

    from concourse.bass_utils import run_bass_kernel_spmd

    xs = _shard(np.asarray(x, dtype=np.float32))
    ys = _shard(np.asarray(y, dtype=np.float32))
    in_maps = [{"x": xs[i], "y": ys[i]} for i in range(N_CORES)]
    res = run_bass_kernel_spmd(nc, in_maps, core_ids=list(range(N_CORES)))

    total = np.float64(0.0)
    for r in res.results:
        o = r["o"].astype(np.float64)
        total += o[:, 0::2].sum()
        total -= ALPHA * o[:, 1::2].sum()
    return np.float32(total)


# revision 30
# speedup vs baseline: 1.1046x; 1.0021x over previous
"""Masked-loss kernel for nn_MLoss_9715216024200 on 8 Trainium2 NeuronCores.

loss = sum(where(y[...,0]>0.5, (y-x)^2 - a*x^2, 0)) + a*sum(x[...,0]^2)
with x,y f32 (256, 10647, 5); output is a f32 scalar.

Sharding: flatten both tensors to cells (5 contiguous f32 each), pad with
256 zero-cells (mathematically neutral: y0=0 -> mask 0, x=0 -> no bg term),
reshape to (8 cores, 128 partitions, 2662 cells).  Each core streams its
13 MiB at the ~358 GB/s HBM roofline while three compute engines split the
elementwise work (each under the ~38 us DMA time):

  per tile:
    GpSimd: m5  = bf16(y0 > 0.5) replicated to all 5 features (contiguous)
            xs0 = bf16(sqrt(a)*x0)  -> tail slice of the dmx tile
    DVE:    d   = y - x   (f32 1x, bf16 out)
            dm  = d * m5  (bf16 2x) -> head slice of dmx
            xm  = x * m5  (mixed 1x, bf16 out)
    ScalarE (Square + accum_out, fp32 accumulate):
            acc1[t] = sum(dmx^2) = sum((m*d)^2) + a*sum(x0^2)
            acc2[t] = sum(xm^2)  (unscaled; host applies a)

m*v^2 == (m*v)^2 because m is 0/1, which is what lets ScalarE's fused
Square-accumulate do all reductions.  bf16 intermediates cost ~1e-6
relative error on the final sum.  Host combines:
total = sum(acc1) - a*sum(acc2), in f64 over 8 cores x 128 partitions.
"""
import sys

for _p in ('/opt/trn_rl_repo',):
    if _p in sys.path:
        sys.path.remove(_p)
    sys.path.insert(0, _p)

import numpy as np

B, C, F = 256, 10647, 5
THRESH = 0.5
ALPHA = 0.1
N_CORES = 8
P = 128
CELLS = B * C                      # 2,725,632
CELLS_PER_PART = 2662              # ceil to 8*128*2662 = 2,725,888
PAD_CELLS = N_CORES * P * CELLS_PER_PART - CELLS   # 256
FD = CELLS_PER_PART * F            # 13310 elems per partition per core
# 253-cell tiles amortize DMA and instruction overhead; the tail tiles
# shrink so the post-last-DMA dependency chain is short.
import os as _os
_ts = _os.environ.get('TILE_SIZES', '')
TILE_SIZES = ([int(v) for v in _ts.split(',')] if _ts
              else [253] * 8 + [218, 178, 121, 121])  # sums to CELLS_PER_PART
assert sum(TILE_SIZES) == CELLS_PER_PART
N_TILES = len(TILE_SIZES)
# reduction groups: consecutive tiles sharing one dmx/xm buffer and one
# sq/sq2 pair (amortizes ACT's ~370ns fixed cost per accumulate).
# '2' -> groups of 2 from the front, singletons for leftovers at the end;
# explicit '2,2,2,1,1,...' lists group sizes.
_go = _os.environ.get('GROUP_OF', '1')
if ',' in _go:
    _gsizes = [int(v) for v in _go.split(',')]
else:
    _g = int(_go)
    _gsizes = []
    _rem = N_TILES
    while _rem > 0:
        take = min(_g, _rem)
        _gsizes.append(take)
        _rem -= take
assert sum(_gsizes) == N_TILES, _gsizes
GROUP_OF = _gsizes
N_GROUPS = len(GROUP_OF)
# tile index -> (group index, first tile of group?)
_tile_group = []
for _gi, _gn in enumerate(GROUP_OF):
    for _k in range(_gn):
        _tile_group.append((_gi, _k))
_xmp = _os.environ.get('XM_ON_POOL', '3,6,9,11')
XM_ON_POOL = set(int(v) for v in _xmp.split(',') if v)
TTR_TAIL = int(_os.environ.get('TTR_TAIL', '2'))  # last k tiles: sq2 on DVE
M5_ON_DVE = set(int(v) for v in _os.environ.get('M5_ON_DVE', '0').split(',') if v != '')
# tiles whose mask is built on ACT as sigmoid(4096*(y0-0.5)): exact 0/1 in
# bf16 except within ~0.002 of the threshold (~0.4% of uniform cells, well
# inside the 2e-2 rel-err budget)
M5_ON_ACT = set(int(v) for v in _os.environ.get('M5_ON_ACT', '').split(',') if v != '')
# tiles whose sq2 runs as a DVE scalar_tensor_tensor instead of on ACT
SQ2_ON_DVE = set(int(v) for v in _os.environ.get('SQ2_ON_DVE', '').split(',') if v != '')
# last k tiles: sq1 (the dmx reduction) also on DVE, deferred
SQ1_TAIL = int(_os.environ.get('SQ1_TAIL', '0'))
X0_ON_ACT = _os.environ.get('X0_ON_ACT', '0') == '1'
# kvwb: store via SWDGE kv_writeback prepared mid-stream + trigger_dma at
# the end -- skips the HWDGE descriptor-gen (625ns) and DGE-DMA delay
# (650ns) that a plain dma_start pays after the last accumulate lands.
STORE_MODE = _os.environ.get('STORE_MODE', 'dma')
# tiles whose xs0 runs on ACT (activation Copy with scale) instead of Pool
_x0a = _os.environ.get('X0_ON_ACT_TILES', '')
X0_ON_ACT_TILES = set(int(v) for v in _x0a.split(',') if v != '')
BUFS = [int(v) for v in _os.environ.get('BUFS', '8,8,8,4').split(',')]

_compiled = None


def _build():
    from contextlib import ExitStack
    import concourse.tile as tile
    from concourse import bacc, mybir

    sqa = float(np.sqrt(ALPHA))

    nc = bacc.Bacc("TRN2", target_bir_lowering=False, debug=False,
                   enable_asserts=True, num_devices=N_CORES)
    x_d = nc.dram_tensor("x", [P, FD], mybir.dt.float32, kind="ExternalInput").ap()
    y_d = nc.dram_tensor("y", [P, FD], mybir.dt.float32, kind="ExternalInput").ap()
    # [batch=1, d_head_inner=P, d_head_outer=1, n_ctx=2*N_GROUPS]: the 4D
    # shape kv_writeback wants; the plain-store path slices it back to 2D
    o_d = nc.dram_tensor("o", [1, P, 1, 2 * N_GROUPS], mybir.dt.float32,
                         kind="ExternalOutput").ap()

    f32 = mybir.dt.float32
    bf16 = mybir.dt.bfloat16
    Sq = mybir.ActivationFunctionType.Square
    Alu = mybir.AluOpType

    with tile.TileContext(nc) as tc, ExitStack() as ctx:
        xp = ctx.enter_context(tc.tile_pool(name="x", bufs=BUFS[0]))
        yp = ctx.enter_context(tc.tile_pool(name="y", bufs=BUFS[1]))
        wp = ctx.enter_context(tc.tile_pool(name="work", bufs=BUFS[2]))
        sp = ctx.enter_context(tc.tile_pool(name="scratch", bufs=BUFS[3]))
        ap_ = ctx.enter_context(tc.tile_pool(name="acc", bufs=1))

        # interleaved acc layout: columns [2g, 2g+1] = (dm-side, xm-side) of
        # group g, so each group's pair can be stored as soon as it's ready
        acc = ap_.tile([P, 2 * N_GROUPS], f32)

        if STORE_MODE == 'kvwb':
            idx0 = ap_.tile([P, 1], mybir.dt.int32)
            nc.gpsimd.memset(idx0[:], 0)
            store_sem = nc.alloc_semaphore("store_dma_sem")

        K_STEEP = 4096.0
        if M5_ON_ACT:
            bp = ctx.enter_context(tc.tile_pool(name="bias", bufs=1))
            sig_bias = bp.tile([P, 1], f32)
            nc.gpsimd.memset(sig_bias[:], -K_STEEP * THRESH)

        tail_ttr = []
        tail_sq1 = []
        off = 0
        gdmx = gxm = None
        gdoff = gxoff = 0
        for t, cells in enumerate(TILE_SIZES):
            fd = cells * F
            g, k_in_g = _tile_group[t]
            gn = GROUP_OF[g]
            gcells = sum(TILE_SIZES[t - k_in_g:t - k_in_g + gn])
            xt = xp.tile([P, fd], f32, tag="xt")
            yt = yp.tile([P, fd], f32, tag="yt")
            sl = slice(off, off + fd)
            off += fd
            nc.sync.dma_start(yt[:], y_d[:, sl])
            # tile 0's x descgen on ACT's HWDGE port, parallel with y0's on SP
            (nc.scalar if t == 0 and X0_ON_ACT else nc.sync).dma_start(
                xt[:], x_d[:, sl])

            if k_in_g == 0:
                gdmx = wp.tile([P, (gcells * F) + gcells], bf16, tag="dmx")
                gxm = wp.tile([P, gcells * F], bf16, tag="xmg")
                gdoff = gxoff = 0

            # bf16 mask replicated to all 5 features (contiguous); emitted
            # before xs0 because dm (critical path) waits on it.  Tile 0's
            # mask runs on DVE: at the pipeline head Pool's slow broadcast
            # would gate the first dm (and ACT's start)
            m5 = wp.tile([P, fd], bf16, tag="m5")
            y0b = yt[:, 0::F].unsqueeze(2).broadcast_to((P, cells, F))
            m5_eng = nc.vector if t in M5_ON_DVE else nc.gpsimd
            m5_eng.tensor_scalar(
                m5[:].rearrange("p (k f) -> p k f", f=F), y0b,
                THRESH, None, op0=Alu.is_gt)

            # GpSimd: xs0 = sqrt(a)*x0 into the tail slice of this tile's
            # dmx range
            nc.gpsimd.tensor_scalar(
                gdmx[:, gdoff + fd:gdoff + fd + cells], xt[:, 0::F],
                sqa, None, op0=Alu.mult)

            # DVE: d = y - x (bf16 out), dm = d*m5 (bf16 2x), xm = x*m5
            dt_ = wp.tile([P, fd], bf16, tag="d")
            nc.vector.tensor_tensor(dt_[:], yt[:], xt[:], op=Alu.subtract)
            nc.vector.tensor_tensor(gdmx[:, gdoff:gdoff + fd], dt_[:], m5[:],
                                    op=Alu.mult)
            xm_eng = nc.gpsimd if t in XM_ON_POOL else nc.vector
            xm_eng.tensor_tensor(gxm[:, gxoff:gxoff + fd], xt[:], m5[:],
                                 op=Alu.mult)
            gdoff += fd + cells
            gxoff += fd

            if k_in_g == gn - 1:
                # group complete: fused square + row-sum over the group
                # buffers (ScalarE, or deferred DVE stt at the tail)
                gc = gcells
                if g >= N_GROUPS - SQ1_TAIL:
                    tail_sq1.append((g, gdmx, gc * F + gc))
                else:
                    sq = sp.tile([P, gc * F + gc], bf16, tag="sq")
                    nc.scalar.activation(sq[:], gdmx[:], Sq,
                                         accum_out=acc[:, 2 * g:2 * g + 1])
                if g >= N_GROUPS - TTR_TAIL:
                    tail_ttr.append((g, gxm, gc))
                elif g in SQ2_ON_DVE:
                    sq2 = sp.tile([P, gc * F], bf16, tag="sq2")
                    nc.vector.scalar_tensor_tensor(
                        sq2[:], gxm[:], 1.0, gxm[:],
                        op0=Alu.mult, op1=Alu.mult,
                        accum_out=acc[:, 2 * g + 1:2 * g + 2])
                else:
                    sq2 = sp.tile([P, gc * F], bf16, tag="sq2")
                    nc.scalar.activation(sq2[:], gxm[:], Sq,
                                         accum_out=acc[:, 2 * g + 1:2 * g + 2])

        for (g, gdmx, n) in tail_sq1:
            s1 = sp.tile([P, n], bf16, tag="sq")
            nc.vector.scalar_tensor_tensor(
                s1[:], gdmx[:], 1.0, gdmx[:],
                op0=Alu.mult, op1=Alu.mult, accum_out=acc[:, 2 * g:2 * g + 1])

        for (g, gxm, gc) in tail_ttr:
            # xm * 1 * xm summed per row == sum(xm^2); runs on DVE
            # (scalar_tensor_tensor is Pool-invalid but DVE-valid on HW)
            sq2 = sp.tile([P, gc * F], bf16, tag="sq2")
            nc.vector.scalar_tensor_tensor(
                sq2[:], gxm[:], 1.0, gxm[:],
                op0=Alu.mult, op1=Alu.mult, accum_out=acc[:, 2 * g + 1:2 * g + 2])

        if STORE_MODE == 'kvwb':
            # descriptors generated mid-stream; the acc data-dependency moves
            # to the trigger, which then only pays SEQ decode + DMA transfer.
            # No explicit completion wait: the SWDGE queue drains before the
            # NEFF completes, and the cost model counts the sem-prop.
            nc.gpsimd.kv_writeback(
                o_d,
                acc[:].rearrange("p (a b n) -> p a b n", a=1, b=1),
                idx0[:],
                prepare_only=True, sem=store_sem)
            nc.gpsimd.trigger_dma(count=None)
        else:
            nc.sync.dma_start(o_d[0, :, 0, :], acc[:])

    nc.compile()
    return nc


def _shard(a: np.ndarray) -> list[np.ndarray]:
    flat = a.reshape(-1)
    pad = np.zeros(PAD_CELLS * F, dtype=a.dtype)
    flat = np.concatenate([flat, pad])
    per_core = flat.reshape(N_CORES, P, FD)
    return [np.ascontiguousarray(per_core[i]) for i in range(N_CORES)]


def kernel(x: np.ndarray, y: np.ndarray) -> np.ndarray:
    global _compiled
    if _compiled is None:
        _compiled = _build()
    nc = _compiled

    from concourse.bass_utils import run_bass_kernel_spmd

    xs = _shard(np.asarray(x, dtype=np.float32))
    ys = _shard(np.asarray(y, dtype=np.float32))
    in_maps = [{"x": xs[i], "y": ys[i]} for i in range(N_CORES)]
    res = run_bass_kernel_spmd(nc, in_maps, core_ids=list(range(N_CORES)))

    total = np.float64(0.0)
    for r in res.results:
        o = r["o"].astype(np.float64).reshape(P, 2 * N_GROUPS)
        total += o[:, 0::2].sum()
        total -= ALPHA * o[:, 1::2].sum()
    return np.float32(total)


# revision 33
# speedup vs baseline: 1.1127x; 1.0073x over previous
"""Masked-loss kernel for nn_MLoss_9715216024200 on 8 Trainium2 NeuronCores.

loss = sum(where(y[...,0]>0.5, (y-x)^2 - a*x^2, 0)) + a*sum(x[...,0]^2)
with x,y f32 (256, 10647, 5); output is a f32 scalar.

Sharding: flatten both tensors to cells (5 contiguous f32 each), pad with
256 zero-cells (mathematically neutral: y0=0 -> mask 0, x=0 -> no bg term),
reshape to (8 cores, 128 partitions, 2662 cells).  Each core streams its
13 MiB at the ~358 GB/s HBM roofline while three compute engines split the
elementwise work (each under the ~38 us DMA time):

  per tile:
    GpSimd: m5  = bf16(y0 > 0.5) replicated to all 5 features (contiguous)
            xs0 = bf16(sqrt(a)*x0)  -> tail slice of the dmx tile
    DVE:    d   = y - x   (f32 1x, bf16 out)
            dm  = d * m5  (bf16 2x) -> head slice of dmx
            xm  = x * m5  (mixed 1x, bf16 out)
    ScalarE (Square + accum_out, fp32 accumulate):
            acc1[t] = sum(dmx^2) = sum((m*d)^2) + a*sum(x0^2)
            acc2[t] = sum(xm^2)  (unscaled; host applies a)

m*v^2 == (m*v)^2 because m is 0/1, which is what lets ScalarE's fused
Square-accumulate do all reductions.  bf16 intermediates cost ~1e-6
relative error on the final sum.  Host combines:
total = sum(acc1) - a*sum(acc2), in f64 over 8 cores x 128 partitions.
"""
import sys

for _p in ('/opt/trn_rl_repo',):
    if _p in sys.path:
        sys.path.remove(_p)
    sys.path.insert(0, _p)

import numpy as np

B, C, F = 256, 10647, 5
THRESH = 0.5
ALPHA = 0.1
N_CORES = 8
P = 128
CELLS = B * C                      # 2,725,632
CELLS_PER_PART = 2662              # ceil to 8*128*2662 = 2,725,888
PAD_CELLS = N_CORES * P * CELLS_PER_PART - CELLS   # 256
FD = CELLS_PER_PART * F            # 13310 elems per partition per core
# 253-cell tiles amortize DMA and instruction overhead; the tail tiles
# shrink so the post-last-DMA dependency chain is short.
import os as _os
_ts = _os.environ.get('TILE_SIZES', '')
TILE_SIZES = ([int(v) for v in _ts.split(',')] if _ts
              else [127] * 16 + [218, 170, 121, 121])  # sums to CELLS_PER_PART
assert sum(TILE_SIZES) == CELLS_PER_PART
N_TILES = len(TILE_SIZES)
# reduction groups: consecutive tiles sharing one dmx/xm buffer and one
# sq/sq2 pair (amortizes ACT's ~370ns fixed cost per accumulate).
# '2' -> groups of 2 from the front, singletons for leftovers at the end;
# explicit '2,2,2,1,1,...' lists group sizes.
_go = _os.environ.get('GROUP_OF', '3,3,3,3,2,2,1,1,1,1')
if ',' in _go:
    _gsizes = [int(v) for v in _go.split(',')]
else:
    _g = int(_go)
    _gsizes = []
    _rem = N_TILES
    while _rem > 0:
        take = min(_g, _rem)
        _gsizes.append(take)
        _rem -= take
assert sum(_gsizes) == N_TILES, _gsizes
GROUP_OF = _gsizes
N_GROUPS = len(GROUP_OF)
# tile index -> (group index, first tile of group?)
_tile_group = []
for _gi, _gn in enumerate(GROUP_OF):
    for _k in range(_gn):
        _tile_group.append((_gi, _k))
_xmp = _os.environ.get('XM_ON_POOL', '3,7,11,15,17,19')
XM_ON_POOL = set(int(v) for v in _xmp.split(',') if v)
TTR_TAIL = int(_os.environ.get('TTR_TAIL', '2'))  # last k tiles: sq2 on DVE
M5_ON_DVE = set(int(v) for v in _os.environ.get('M5_ON_DVE', '0').split(',') if v != '')
# tiles whose mask is built on ACT as sigmoid(4096*(y0-0.5)): exact 0/1 in
# bf16 except within ~0.002 of the threshold (~0.4% of uniform cells, well
# inside the 2e-2 rel-err budget)
M5_ON_ACT = set(int(v) for v in _os.environ.get('M5_ON_ACT', '').split(',') if v != '')
# tiles whose sq2 runs as a DVE scalar_tensor_tensor instead of on ACT
SQ2_ON_DVE = set(int(v) for v in _os.environ.get('SQ2_ON_DVE', '').split(',') if v != '')
# last k tiles: sq1 (the dmx reduction) also on DVE, deferred
SQ1_TAIL = int(_os.environ.get('SQ1_TAIL', '0'))
X0_ON_ACT = _os.environ.get('X0_ON_ACT', '0') == '1'
# kvwb: store via SWDGE kv_writeback prepared mid-stream + trigger_dma at
# the end -- skips the HWDGE descriptor-gen (625ns) and DGE-DMA delay
# (650ns) that a plain dma_start pays after the last accumulate lands.
STORE_MODE = _os.environ.get('STORE_MODE', 'dma')
# tiles whose xs0 runs on ACT (activation Copy with scale) instead of Pool
_x0a = _os.environ.get('X0_ON_ACT_TILES', '')
X0_ON_ACT_TILES = set(int(v) for v in _x0a.split(',') if v != '')
BUFS = [int(v) for v in _os.environ.get('BUFS', '8,8,8,4').split(',')]

_compiled = None


def _build():
    from contextlib import ExitStack
    import concourse.tile as tile
    from concourse import bacc, mybir

    sqa = float(np.sqrt(ALPHA))

    nc = bacc.Bacc("TRN2", target_bir_lowering=False, debug=False,
                   enable_asserts=True, num_devices=N_CORES)
    x_d = nc.dram_tensor("x", [P, FD], mybir.dt.float32, kind="ExternalInput").ap()
    y_d = nc.dram_tensor("y", [P, FD], mybir.dt.float32, kind="ExternalInput").ap()
    # [batch=1, d_head_inner=P, d_head_outer=1, n_ctx=2*N_GROUPS]: the 4D
    # shape kv_writeback wants; the plain-store path slices it back to 2D
    o_d = nc.dram_tensor("o", [1, P, 1, 2 * N_GROUPS], mybir.dt.float32,
                         kind="ExternalOutput").ap()

    f32 = mybir.dt.float32
    bf16 = mybir.dt.bfloat16
    Sq = mybir.ActivationFunctionType.Square
    Alu = mybir.AluOpType

    with tile.TileContext(nc) as tc, ExitStack() as ctx:
        xp = ctx.enter_context(tc.tile_pool(name="x", bufs=BUFS[0]))
        yp = ctx.enter_context(tc.tile_pool(name="y", bufs=BUFS[1]))
        wp = ctx.enter_context(tc.tile_pool(name="work", bufs=BUFS[2]))
        sp = ctx.enter_context(tc.tile_pool(name="scratch", bufs=BUFS[3]))
        ap_ = ctx.enter_context(tc.tile_pool(name="acc", bufs=1))

        # interleaved acc layout: columns [2g, 2g+1] = (dm-side, xm-side) of
        # group g, so each group's pair can be stored as soon as it's ready
        acc = ap_.tile([P, 2 * N_GROUPS], f32)

        if STORE_MODE == 'kvwb':
            idx0 = ap_.tile([P, 1], mybir.dt.int32)
            nc.gpsimd.memset(idx0[:], 0)
            store_sem = nc.alloc_semaphore("store_dma_sem")

        K_STEEP = 4096.0
        if M5_ON_ACT:
            bp = ctx.enter_context(tc.tile_pool(name="bias", bufs=1))
            sig_bias = bp.tile([P, 1], f32)
            nc.gpsimd.memset(sig_bias[:], -K_STEEP * THRESH)

        tail_ttr = []
        tail_sq1 = []
        off = 0
        gdmx = gxm = None
        gdoff = gxoff = 0
        for t, cells in enumerate(TILE_SIZES):
            fd = cells * F
            g, k_in_g = _tile_group[t]
            gn = GROUP_OF[g]
            gcells = sum(TILE_SIZES[t - k_in_g:t - k_in_g + gn])
            xt = xp.tile([P, fd], f32, tag="xt")
            yt = yp.tile([P, fd], f32, tag="yt")
            sl = slice(off, off + fd)
            off += fd
            nc.sync.dma_start(yt[:], y_d[:, sl])
            # tile 0's x descgen on ACT's HWDGE port, parallel with y0's on SP
            (nc.scalar if t == 0 and X0_ON_ACT else nc.sync).dma_start(
                xt[:], x_d[:, sl])

            if k_in_g == 0:
                gdmx = wp.tile([P, (gcells * F) + gcells], bf16, tag="dmx")
                gxm = wp.tile([P, gcells * F], bf16, tag="xmg")
                gdoff = gxoff = 0

            # bf16 mask replicated to all 5 features (contiguous); emitted
            # before xs0 because dm (critical path) waits on it.  Tile 0's
            # mask runs on DVE: at the pipeline head Pool's slow broadcast
            # would gate the first dm (and ACT's start)
            m5 = wp.tile([P, fd], bf16, tag="m5")
            y0b = yt[:, 0::F].unsqueeze(2).broadcast_to((P, cells, F))
            m5_eng = nc.vector if t in M5_ON_DVE else nc.gpsimd
            m5_eng.tensor_scalar(
                m5[:].rearrange("p (k f) -> p k f", f=F), y0b,
                THRESH, None, op0=Alu.is_gt)

            # GpSimd: xs0 = sqrt(a)*x0 into the tail slice of this tile's
            # dmx range
            nc.gpsimd.tensor_scalar(
                gdmx[:, gdoff + fd:gdoff + fd + cells], xt[:, 0::F],
                sqa, None, op0=Alu.mult)

            # DVE: d = y - x (bf16 out), dm = d*m5 (bf16 2x), xm = x*m5
            dt_ = wp.tile([P, fd], bf16, tag="d")
            nc.vector.tensor_tensor(dt_[:], yt[:], xt[:], op=Alu.subtract)
            nc.vector.tensor_tensor(gdmx[:, gdoff:gdoff + fd], dt_[:], m5[:],
                                    op=Alu.mult)
            xm_eng = nc.gpsimd if t in XM_ON_POOL else nc.vector
            xm_eng.tensor_tensor(gxm[:, gxoff:gxoff + fd], xt[:], m5[:],
                                 op=Alu.mult)
            gdoff += fd + cells
            gxoff += fd

            if k_in_g == gn - 1:
                # group complete: fused square + row-sum over the group
                # buffers (ScalarE, or deferred DVE stt at the tail)
                gc = gcells
                if g >= N_GROUPS - SQ1_TAIL:
                    tail_sq1.append((g, gdmx, gc * F + gc))
                else:
                    sq = sp.tile([P, gc * F + gc], bf16, tag="sq")
                    nc.scalar.activation(sq[:], gdmx[:], Sq,
                                         accum_out=acc[:, 2 * g:2 * g + 1])
                if g >= N_GROUPS - TTR_TAIL:
                    tail_ttr.append((g, gxm, gc))
                elif g in SQ2_ON_DVE:
                    sq2 = sp.tile([P, gc * F], bf16, tag="sq2")
                    nc.vector.scalar_tensor_tensor(
                        sq2[:], gxm[:], 1.0, gxm[:],
                        op0=Alu.mult, op1=Alu.mult,
                        accum_out=acc[:, 2 * g + 1:2 * g + 2])
                else:
                    sq2 = sp.tile([P, gc * F], bf16, tag="sq2")
                    nc.scalar.activation(sq2[:], gxm[:], Sq,
                                         accum_out=acc[:, 2 * g + 1:2 * g + 2])

        for (g, gdmx, n) in tail_sq1:
            s1 = sp.tile([P, n], bf16, tag="sq")
            nc.vector.scalar_tensor_tensor(
                s1[:], gdmx[:], 1.0, gdmx[:],
                op0=Alu.mult, op1=Alu.mult, accum_out=acc[:, 2 * g:2 * g + 1])

        for (g, gxm, gc) in tail_ttr:
            # xm * 1 * xm summed per row == sum(xm^2); runs on DVE
            # (scalar_tensor_tensor is Pool-invalid but DVE-valid on HW)
            sq2 = sp.tile([P, gc * F], bf16, tag="sq2")
            nc.vector.scalar_tensor_tensor(
                sq2[:], gxm[:], 1.0, gxm[:],
                op0=Alu.mult, op1=Alu.mult, accum_out=acc[:, 2 * g + 1:2 * g + 2])

        if STORE_MODE == 'kvwb':
            # descriptors generated mid-stream; the acc data-dependency moves
            # to the trigger, which then only pays SEQ decode + DMA transfer.
            # No explicit completion wait: the SWDGE queue drains before the
            # NEFF completes, and the cost model counts the sem-prop.
            nc.gpsimd.kv_writeback(
                o_d,
                acc[:].rearrange("p (a b n) -> p a b n", a=1, b=1),
                idx0[:],
                prepare_only=True, sem=store_sem)
            nc.gpsimd.trigger_dma(count=None)
        else:
            nc.sync.dma_start(o_d[0, :, 0, :], acc[:])

    nc.compile()
    return nc


def _shard(a: np.ndarray) -> list[np.ndarray]:
    flat = a.reshape(-1)
    pad = np.zeros(PAD_CELLS * F, dtype=a.dtype)
    flat = np.concatenate([flat, pad])
    per_core = flat.reshape(N_CORES, P, FD)
    return [np.ascontiguousarray(per_core[i]) for i in range(N_CORES)]


def kernel(x: np.ndarray, y: np.ndarray) -> np.ndarray:
    global _compiled
    if _compiled is None:
        _compiled = _build()
    nc = _compiled

    from concourse.bass_utils import run_bass_kernel_spmd

    xs = _shard(np.asarray(x, dtype=np.float32))
    ys = _shard(np.asarray(y, dtype=np.float32))
    in_maps = [{"x": xs[i], "y": ys[i]} for i in range(N_CORES)]
    res = run_bass_kernel_spmd(nc, in_maps, core_ids=list(range(N_CORES)))

    total = np.float64(0.0)
    for r in res.results:
        o = r["o"].astype(np.float64).reshape(P, 2 * N_GROUPS)
        total += o[:, 0::2].sum()
        total -= ALPHA * o[:, 1::2].sum()
    return np.float32(total)
